# revision 42
# baseline (speedup 1.0000x reference)
# Multi-headed attention (B=2, A=6, S=1024, E=256, d_model=512, H=8, DK=64)
# distributed over 8 NeuronCores.
#
# Decomposition: the 12 (batch, agent) pairs are each split into two
# "quad-tasks" of 4 heads (d_model halves), giving 24 tasks; each core runs
# 3 tasks (perfect balance, no duplicated FLOPs: QKV projections split
# cleanly along the head dim, the output projection's head contraction is
# summed on the host).
#
# Per-task device pipeline (no on-device transposes anywhere):
#   inputs arrive host-pre-transposed as x^T [E, S].
#   QT = Wq_t^T @ q^T   [F=256, S]      (lhsT = Wq_t, rhs = q^T)
#   KT = Wk_t^T @ k^T   [F=256, S]
#   V  = (v^T)^T @ Wv_t [S, F]          (lhsT = v^T slice, rhs = Wv_t)
#   per head h (64 rows of QT/KT):
#     scoresT[k, q] = K_h @ Q_h^T       (lhsT = KT_h slice, rhs = QT_h slice;
#                                        both 512-chunks land in one 2-bank
#                                        PSUM tile)
#     pT = exp(scoresT / 8)             (ONE [128,1024] ACT instr per k-tile;
#                                        no max subtraction -- scores O(1))
#     xT[65, S]  = [V_h | 1]^T @ pT     (row 64 = softmax denominators)
#     outT_h = Wo_h^T @ xT[0:64]        (RAW, unnormalized)
#   ship outT_h [4, 256, S] and the denominators [4, S]; the host divides
#   (normalization commutes with the per-head linear) and sums heads.
import numpy as np

import concourse.bass as bass
from concourse import bacc
import concourse.mybir as mybir
from concourse.tile import TileContext
from concourse.bass_utils import run_bass_kernel_spmd
from contextlib import ExitStack

B, A, S, E = 2, 6, 1024, 256
DMODEL, H, DK = 512, 8, 64
F = 256                 # per-task projection width (4 heads x 64)
OUTD = 256              # output dim (q_dim)
NT = 3                  # tasks per core
NCORES = 8
P = 128
NPAIR = B * A           # 12
CHUNK = 512             # Sq chunk (one PSUM bank of f32)


def build_nc(n_tasks=NT):
    f32 = mybir.dt.float32
    bf16 = mybir.dt.bfloat16
    ADD = mybir.AluOpType.add
    EXP = mybir.ActivationFunctionType.Exp

    nc = bacc.Bacc(None, target_bir_lowering=False, debug=False)
    qT_d = nc.declare_dram_parameter("qT", [n_tasks, E, S], bf16, isOutput=False)
    kT_d = nc.declare_dram_parameter("kT", [n_tasks, E, S], bf16, isOutput=False)
    vT_d = nc.declare_dram_parameter("vT", [n_tasks, E, S], bf16, isOutput=False)
    wq_d = nc.declare_dram_parameter("wq", [n_tasks, E, F], bf16, isOutput=False)
    wk_d = nc.declare_dram_parameter("wk", [n_tasks, E, F], bf16, isOutput=False)
    wv_d = nc.declare_dram_parameter("wv", [n_tasks, E, F], bf16, isOutput=False)
    wo_d = nc.declare_dram_parameter("wo", [n_tasks, 4, DK, OUTD], bf16, isOutput=False)
    bq_d = nc.declare_dram_parameter("bq", [n_tasks, F], f32, isOutput=False)
    bk_d = nc.declare_dram_parameter("bk", [n_tasks, F], f32, isOutput=False)
    bv_d = nc.declare_dram_parameter("bv", [n_tasks, F], f32, isOutput=False)
    out_d = nc.declare_dram_parameter("out", [n_tasks, 4, 2, P, S], bf16, isOutput=True)
    den_d = nc.declare_dram_parameter("den", [n_tasks, 4, S], bf16, isOutput=True)

    with TileContext(nc) as tc, ExitStack() as ctx:
        inbuf = ctx.enter_context(tc.tile_pool(name="inbuf", bufs=2))
        wbuf = ctx.enter_context(tc.tile_pool(name="wbuf", bufs=2))
        proj = ctx.enter_context(tc.tile_pool(name="proj", bufs=2))
        ptbuf = ctx.enter_context(tc.tile_pool(name="ptbuf", bufs=3))
        xnbuf = ctx.enter_context(tc.tile_pool(name="xnbuf", bufs=3))
        obuf = ctx.enter_context(tc.tile_pool(name="obuf", bufs=2))
        psS = ctx.enter_context(tc.tile_pool(name="psS", bufs=2, space="PSUM"))
        psP = ctx.enter_context(tc.tile_pool(name="psP", bufs=2, space="PSUM"))
        psX = ctx.enter_context(tc.tile_pool(name="psX", bufs=2, space="PSUM"))

        def load_task(t, spread=False):
            """Issue task t's input DMAs (first-consumer order). With
            spread=True (cold start) the loads fan out over four engine
            queues so the transfers land in parallel."""
            qT_sb = inbuf.tile([P, 2, S], bf16, tag="qT", name="qT_sb")
            kT_sb = inbuf.tile([P, 2, S], bf16, tag="kT", name="kT_sb")
            vT_sb = inbuf.tile([P, 2, S], bf16, tag="vT", name="vT_sb")
            wq_sb = wbuf.tile([P, 2, F], bf16, tag="wq", name="wq_sb")
            wk_sb = wbuf.tile([P, 2, F], bf16, tag="wk", name="wk_sb")
            wv_sb = wbuf.tile([P, 2, F], bf16, tag="wv", name="wv_sb")
            wo_sb = wbuf.tile([DK, 4, OUTD], bf16, tag="wo", name="wo_sb")
            bq_sb = wbuf.tile([P, 2], f32, tag="bq", name="bq_sb")
            bk_sb = wbuf.tile([P, 2], f32, tag="bk", name="bk_sb")
            bv_bc = wbuf.tile([P, F], f32, tag="bvbc", name="bv_bc")
            vsb = proj.tile([P, 8, 4, DK + 1], bf16, tag="vsb", name="vsb")
            if spread:
                queues = [nc.sync, nc.scalar, nc.gpsimd]
            else:
                queues = [nc.sync]
            xfers = [
                (qT_sb, qT_d[t].rearrange("(e p) s -> p e s", p=P)),
                (wq_sb, wq_d[t].rearrange("(e p) f -> p e f", p=P)),
                (kT_sb, kT_d[t].rearrange("(e p) s -> p e s", p=P)),
                (wk_sb, wk_d[t].rearrange("(e p) f -> p e f", p=P)),
                (vT_sb, vT_d[t].rearrange("(e p) s -> p e s", p=P)),
                (wv_sb, wv_d[t].rearrange("(e p) f -> p e f", p=P)),
                (bq_sb, bq_d[t].rearrange("(e p) -> p e", p=P)),
                (bk_sb, bk_d[t].rearrange("(e p) -> p e", p=P)),
                (bv_bc, bv_d[t].partition_broadcast(P)),
                (wo_sb, wo_d[t].rearrange("h p m -> p h m")),
            ]
            for i, (dst, src) in enumerate(xfers):
                queues[i % len(queues)].dma_start(out=dst, in_=src)
            nc.gpsimd.memset(vsb[:, :, :, DK : DK + 1], 1.0)
            return qT_sb, kT_sb, vT_sb, wq_sb, wk_sb, wv_sb, wo_sb, bq_sb, bk_sb, bv_bc, vsb

        def qk_proj(ld, dsts, eo, startup=False):
            """Q and K projections for one F tile (eo). At cold start the
            idle scores pool hosts the accumulators so Q and K don't
            serialize on the 2-buffer projection pool."""
            qT_sb, kT_sb, vT_sb, wq_sb, wk_sb, wv_sb, wo_sb, bq_sb, bk_sb, bv_bc, vsb = ld
            qproj, kproj = dsts
            for dst, srct, w_sb, b_sb in (
                (qproj, qT_sb, wq_sb, bq_sb),
                (kproj, kT_sb, wk_sb, bk_sb),
            ):
                if startup:
                    ps2 = psS.tile([P, 2 * CHUNK], f32, tag="pss", name="ps2")
                    ps = [ps2[:, 0:CHUNK], ps2[:, CHUNK : 2 * CHUNK]]
                else:
                    ps = [
                        psP.tile([P, CHUNK], f32, tag="psp", name="ps")
                        for _ in range(2)
                    ]
                for ek in range(2):          # contraction tile over E
                    for n in range(2):       # S chunk (reuses the weights)
                        nc.tensor.matmul(
                            ps[n],
                            lhsT=w_sb[:, ek, 128 * eo : 128 * eo + 128],
                            rhs=srct[:, ek, CHUNK * n : CHUNK * (n + 1)],
                            start=(ek == 0),
                            stop=(ek == 1),
                        )
                for n in range(2):
                    nc.vector.tensor_tensor(
                        out=dst[:, eo, CHUNK * n : CHUNK * (n + 1)],
                        in0=ps[n],
                        in1=b_sb[:, eo : eo + 1].to_broadcast((P, CHUNK)),
                        op=ADD,
                    )

        def v_proj(ld, m):
            """V projection for one S tile (m)."""
            qT_sb, kT_sb, vT_sb, wq_sb, wk_sb, wv_sb, wo_sb, bq_sb, bk_sb, bv_bc, vsb = ld
            psv = psP.tile([P, F], f32, tag="psp", name="psv")
            for ek in range(2):
                nc.tensor.matmul(
                    psv,
                    lhsT=vT_sb[:, ek, 128 * m : 128 * m + 128],
                    rhs=wv_sb[:, ek, :],
                    start=(ek == 0),
                    stop=(ek == 1),
                )
            nc.vector.tensor_tensor(
                out=vsb[:, m, :, 0:DK],
                in0=psv.rearrange("p (h d) -> p h d", h=4),
                in1=bv_bc.rearrange("p (h d) -> p h d", h=4),
                op=ADD,
            )

        qkps = {}

        def qk_group(ld, dsts, eo, step):
            """One weight-group of the Q/K projection: step 0..3 =
            (q,ek0), (q,ek1+evac), (k,ek0), (k,ek1+evac)."""
            qT_sb, kT_sb, vT_sb, wq_sb, wk_sb, wv_sb, wo_sb, bq_sb, bk_sb, bv_bc, vsb = ld
            qproj, kproj = dsts
            di, ek = step // 2, step % 2
            dst, srct, w_sb, b_sb = (
                (qproj, qT_sb, wq_sb, bq_sb),
                (kproj, kT_sb, wk_sb, bk_sb),
            )[di]
            if ek == 0:
                qkps[(di, eo)] = [
                    psP.tile([P, CHUNK], f32, tag="psp", name="ps")
                    for _ in range(2)
                ]
            ps = qkps[(di, eo)]
            for n in range(2):
                nc.tensor.matmul(
                    ps[n],
                    lhsT=w_sb[:, ek, 128 * eo : 128 * eo + 128],
                    rhs=srct[:, ek, CHUNK * n : CHUNK * (n + 1)],
                    start=(ek == 0),
                    stop=(ek == 1),
                )
            if ek == 1:
                for n in range(2):
                    nc.vector.tensor_tensor(
                        out=dst[:, eo, CHUNK * n : CHUNK * (n + 1)],
                        in0=ps[n],
                        in1=b_sb[:, eo : eo + 1].to_broadcast((P, CHUNK)),
                        op=ADD,
                    )
                del qkps[(di, eo)]

        def outproj_mo(t, h, xsb, wo_sb, mo, tail=False):
            """Half of the RAW per-head output projection + ship to DRAM."""
            osb = obuf.tile([P, 2, CHUNK], bf16, tag="osb", name="osb")
            if tail:
                # scores pool is idle at the tail; its 2-bank tiles host both
                # chunks so the matmuls never wait on an evacuation
                ps2 = psS.tile([P, 2 * CHUNK], f32, tag="pss", name="pso2")
                pso = [ps2[:, 0:CHUNK], ps2[:, CHUNK : 2 * CHUNK]]
            else:
                pso = [
                    psP.tile([P, CHUNK], f32, tag="psp", name="pso")
                    for _ in range(2)
                ]
            for n in range(2):
                nc.tensor.matmul(
                    pso[n],
                    lhsT=wo_sb[0:DK, h, 128 * mo : 128 * mo + 128],
                    rhs=xsb[0:DK, n, :],
                    start=True,
                    stop=True,
                )
            if tail:
                # alternate DVE/ACT and ship each chunk as soon as it is
                # cast, shortening the serial epilogue
                for n in range(2):
                    if n == 0:
                        nc.scalar.activation(
                            out=osb[:, n, :],
                            in_=pso[n],
                            func=mybir.ActivationFunctionType.Copy,
                        )
                    else:
                        nc.vector.tensor_copy(out=osb[:, n, :], in_=pso[n])
                    q = nc.sync if n == 0 else nc.gpsimd
                    q.dma_start(
                        out=out_d[t, h, mo, :, CHUNK * n : CHUNK * (n + 1)],
                        in_=osb[:, n, :],
                    )
            else:
                for n in range(2):
                    nc.vector.tensor_copy(out=osb[:, n, :], in_=pso[n])
                q = nc.gpsimd if h % 2 == 0 else nc.sync
                q.dma_start(out=out_d[t, h, mo], in_=osb)

        # Warm the PE p-state during the initial input-DMA wait: small
        # matmuls on a memset tile ramp the clock before real work lands.
        warm = wbuf.tile([P, P], bf16, tag="warm", name="warm")
        nc.gpsimd.memset(warm, 0.0)
        warmo = wbuf.tile([1, 32], bf16, tag="warmo", name="warmo")
        # dummy exp pre-loads the ACT function table during the DMA wait
        nc.scalar.activation(out=warmo, in_=warm[0:1, 0:32], func=EXP, scale=0.125)
        for w in range(16):
            psw = psP.tile([P, 64], f32, tag="psp", name="psw")
            nc.tensor.matmul(psw, lhsT=warm, rhs=warm[:, 0:64], start=True, stop=True)

        ld = load_task(0, spread=True)
        qproj0 = proj.tile([P, 2, S], bf16, tag="qproj", name="qproj")
        kproj0 = proj.tile([P, 2, S], bf16, tag="kproj", name="kproj")
        qk_proj(ld, (qproj0, kproj0), 0)   # eo=1 deferred into the h1 loop
        v_proj(ld, 0)
        state = (qproj0, kproj0, ld, ld[10], ld[6])
        pending = None                       # (t, h, xsb, wo_sb) awaiting outproj
        for t in range(n_tasks):
            qproj, kproj, ld, vsb, wo_sb = state
            last = t + 1 >= n_tasks
            for h in range(4):
                e, r0 = h // 2, 64 * (h % 2)
                psx = [
                    psX.tile([P, CHUNK], f32, tag="psx", name="psx")
                    for _ in range(2)
                ]
                for m in range(8):           # Sk tiles
                    pss = psS.tile([P, 2 * CHUNK], f32, tag="pss", name="pss")
                    for n in range(2):
                        nc.tensor.matmul(
                            pss[:, CHUNK * n : CHUNK * (n + 1)],
                            lhsT=kproj[r0 : r0 + 64, e, 128 * m : 128 * m + 128],
                            rhs=qproj[r0 : r0 + 64, e, CHUNK * n : CHUNK * (n + 1)],
                            start=True,
                            stop=True,
                        )
                    pt = ptbuf.tile([P, 2 * CHUNK], bf16, tag="pt", name="pt")
                    nc.scalar.activation(out=pt, in_=pss, func=EXP, scale=0.125)
                    # ---- foreign PE work rides the exp latency window ----
                    if m == 1 and pending is not None:
                        outproj_mo(*pending, 0)
                    if m == 2 and pending is not None:
                        outproj_mo(*pending, 1)
                        pending = None
                    if t == 0:
                        if h == 0 and m < 7:
                            v_proj(ld, m + 1)
                        if h == 1 and m == 1:
                            qk_proj(ld, (qproj, kproj), 1)
                    if h == 0 and m == 1 and not last:
                        ld_next = load_task(t + 1)
                        qproj_n = proj.tile([P, 2, S], bf16, tag="qproj", name="qproj")
                        kproj_n = proj.tile([P, 2, S], bf16, tag="kproj", name="kproj")
                    if not last:
                        if h == 1 and m in (3, 4, 5, 6):
                            qk_group(ld_next, (qproj_n, kproj_n), 0, m - 3)
                        if h == 1 and m == 7:
                            v_proj(ld_next, 0)
                        if h == 2 and m in (3, 4, 5, 6):
                            qk_group(ld_next, (qproj_n, kproj_n), 1, m - 3)
                        if h == 2 and m == 7:
                            v_proj(ld_next, 1)
                        if h == 3 and m == 0:
                            v_proj(ld_next, 2)
                        if h == 3 and m >= 3:
                            v_proj(ld_next, m)
                    # ------------------------------------------------------
                    for n in range(2):
                        nc.tensor.matmul(
                            psx[n][0 : DK + 1, :],
                            lhsT=vsb[:, m, h, :],
                            rhs=pt[:, CHUNK * n : CHUNK * (n + 1)],
                            start=(m == 0),
                            stop=(m == 7),
                        )
                xsb = xnbuf.tile([P, 2, CHUNK], bf16, tag="xsb", name="xsb")
                # split the evacuation across ACT and DVE so the PSUM banks
                # free before the next head's first AV matmul needs them
                nc.scalar.activation(
                    out=xsb[0 : DK + 1, 0, :],
                    in_=psx[0][0 : DK + 1, :],
                    func=mybir.ActivationFunctionType.Copy,
                )
                nc.vector.tensor_copy(
                    out=xsb[0 : DK + 1, 1, :], in_=psx[1][0 : DK + 1, :]
                )
                nc.sync.dma_start(out=den_d[t, h], in_=xsb[DK : DK + 1, :, :])
                pending = (t, h, xsb, wo_sb)
            if not last:
                state = (qproj_n, kproj_n, ld_next, ld_next[10], ld_next[6])
        outproj_mo(*pending, 0, tail=True)
        outproj_mo(*pending, 1, tail=True)

    nc.finalize()
    return nc


_cache = {}


def _get_nc():
    if "nc" not in _cache:
        _cache["nc"] = build_nc()
    return _cache["nc"]


def _tasks_of(c):
    return [NT * c + j for j in range(NT)]


def make_in_maps(query, key, value, Wq, bq, Wk, bk, Wv, bv, Wo, bo):
    import ml_dtypes

    in_dt = ml_dtypes.bfloat16
    f = np.float32
    q = np.asarray(query, f).reshape(NPAIR, S, E)
    k = np.asarray(key, f).reshape(NPAIR, S, E)
    v = np.asarray(value, f).reshape(NPAIR, S, E)
    qT = np.ascontiguousarray(q.transpose(0, 2, 1))
    kT = np.ascontiguousarray(k.transpose(0, 2, 1))
    vT = np.ascontiguousarray(v.transpose(0, 2, 1))
    Wq_, Wk_, Wv_, Wo_ = (np.asarray(w, f) for w in (Wq, Wk, Wv, Wo))
    bq_, bk_, bv_ = (np.asarray(b, f) for b in (bq, bk, bv))

    in_maps = []
    for c in range(NCORES):
        ts = _tasks_of(c)
        pairs = [t // 2 for t in ts]
        sls = [slice(F * (t % 2), F * (t % 2) + F) for t in ts]
        in_maps.append(
            {
                "qT": np.ascontiguousarray(qT[pairs]).astype(in_dt),
                "kT": np.ascontiguousarray(kT[pairs]).astype(in_dt),
                "vT": np.ascontiguousarray(vT[pairs]).astype(in_dt),
                "wq": np.ascontiguousarray(np.stack([Wq_[:, s] for s in sls])).astype(in_dt),
                "wk": np.ascontiguousarray(np.stack([Wk_[:, s] for s in sls])).astype(in_dt),
                "wv": np.ascontiguousarray(np.stack([Wv_[:, s] for s in sls])).astype(in_dt),
                "wo": np.ascontiguousarray(np.stack([Wo_[s, :].reshape(4, DK, OUTD) for s in sls])).astype(in_dt),
                "bq": np.stack([bq_[s] for s in sls]),
                "bk": np.stack([bk_[s] for s in sls]),
                "bv": np.stack([bv_[s] for s in sls]),
            }
        )
    return in_maps


def assemble_output(results, bo):
    out = np.zeros((NPAIR, S, OUTD), np.float32)
    for c in range(NCORES):
        o = np.asarray(results[c]["out"], np.float32)     # [NT, 4, 2, 128, S]
        den = np.asarray(results[c]["den"], np.float32)   # [NT, 4, S]
        for j, t in enumerate(_tasks_of(c)):
            x = o[j].reshape(4, OUTD, S) / den[j][:, None, :]
            out[t // 2] += x.sum(0).T
    out += np.asarray(bo, np.float32)
    return out.reshape(B, A, S, OUTD)


def kernel(query, key, value, Wq, bq, Wk, bk, Wv, bv, Wo, bo):
    import time

    in_maps = make_in_maps(query, key, value, Wq, bq, Wk, bk, Wv, bv, Wo, bo)
    last_err = None
    for _ in range(3):  # the device occasionally reports a transient
        try:            # NRT_EXEC_UNIT_UNRECOVERABLE on a fresh load; retry
            res = run_bass_kernel_spmd(
                _get_nc(), in_maps, core_ids=list(range(NCORES))
            )
            out = assemble_output(res.results, bo)
            if np.isfinite(out).all():
                return out
            last_err = RuntimeError("non-finite output")
        except Exception as e:  # noqa: BLE001
            last_err = e
        time.sleep(2)
    raise last_err
